# revision 6
# baseline (speedup 1.0000x reference)
"""MinCutPool layer on 8 Trainium2 NeuronCores — data-parallel over batch B=8.

Each core processes one graph: x [4096,512], A [4096,4096].
Device computes (per core):
  s      = softmax(x @ W^T + b) * mask            [N, K]
  s_aug  = [s, ones, q],  q[n] = sum_k s[n,k]^2   [N, K+2]
  t      = s_aug^T A                              [K+2, N]   (A streamed, bf16)
  out    = s^T x                                  [K, F]
  ss_aug = s_aug^T s_aug                          [K+2, K+2]
  oa_aug = s_aug^T A s_aug                        [K+2, K+2]
    -> oa_aug[:K,:K] = s^T A s,  trace = mincut numerator
    -> oa_aug[K+1, K] = q^T A 1 = sum_n deg[n] q[n] = mincut denominator
Host finishes: BatchNorm stats over the 8 gathered `out`s, zero-diag +
degree-normalization of out_adj, and the scalar mc + o loss means.
"""

import sys

import numpy as np

if "/opt/trn_rl_repo" not in sys.path:
    sys.path.insert(0, "/opt/trn_rl_repo")

import ml_dtypes

B, N, F, K = 8, 4096, 512, 100
K2 = K + 2          # s columns + ones column + q column
P = 128             # partitions
NT = N // P         # 32 n-tiles
FT = F // P         # 4 f-chunks
EPS = 1e-15
BN_EPS = 1e-5

A_DTYPE = "bfloat16"   # "float32" or "bfloat16" for the adjacency stream
LBLK = 512 if A_DTYPE == "bfloat16" else 256   # A column-block width
LTN = N // LBLK

_CACHE = {}


def _build_program():
    import concourse.bass as bass
    import concourse.mybir as mybir
    import concourse.tile as tile
    from concourse import bacc
    from concourse.masks import make_identity

    dt = mybir.dt
    adt = getattr(dt, A_DTYPE)
    fp32 = dt.float32

    nc = bacc.Bacc("TRN2", target_bir_lowering=False, debug=False,
                   num_devices=B)
    x_d = nc.dram_tensor("x", [N, F], fp32, kind="ExternalInput")
    a_d = nc.dram_tensor("a", [N, N], adt, kind="ExternalInput")
    wt_d = nc.dram_tensor("wt", [F, K], fp32, kind="ExternalInput")
    b_d = nc.dram_tensor("b", [K], fp32, kind="ExternalInput")
    m_d = nc.dram_tensor("m", [N], fp32, kind="ExternalInput")
    out_pre_d = nc.dram_tensor("out_pre", [K, F], fp32, kind="ExternalOutput")
    oa_d = nc.dram_tensor("oa", [K2, K2], fp32, kind="ExternalOutput")
    ss_d = nc.dram_tensor("ss", [K2, K2], fp32, kind="ExternalOutput")

    AX = mybir.AxisListType
    AF = mybir.ActivationFunctionType

    with tile.TileContext(nc) as tc:
        with tc.tile_pool(name="singles", bufs=1) as singles, \
             tc.tile_pool(name="apool", bufs=2) as apool:
            x_sb = singles.tile([P, NT, F], fp32, tag="x_sb")
            s_sb = singles.tile([P, NT, K2], fp32, tag="s_sb")
            s_bf = singles.tile([P, NT, K2], adt, tag="s_bf")
            t_sb = singles.tile([P, N], fp32, tag="t_sb")
            tt_sb = singles.tile([P, NT, K2], fp32, tag="tt_sb")
            wt_sb = singles.tile([P, FT, K], fp32, tag="wt_sb")
            bb_sb = singles.tile([P, K], fp32, tag="bb_sb")
            m_sb = singles.tile([P, NT], fp32, tag="m_sb")
            ident = singles.tile([P, P], fp32, tag="ident")
            out_st = singles.tile([P, F], fp32, tag="out_st")
            oa_st = singles.tile([P, K2], fp32, tag="oa_st")
            ss_st = singles.tile([P, K2], fp32, tag="ss_st")

            make_identity(nc, ident[:, :])
            # bias broadcast to all partitions via partition-step-0 DMA
            bb_ap = bass.AP(tensor=b_d, offset=0, ap=[[0, P], [1, K]])
            nc.sync.dma_start(out=bb_sb[:, :], in_=bb_ap)
            nc.sync.dma_start(out=m_sb[:, :], in_=m_d[:].rearrange("(j p) -> p j", p=P))
            for fc in range(FT):
                nc.sync.dma_start(out=wt_sb[:, fc, :], in_=wt_d[fc * P:(fc + 1) * P, :])
            for j in range(NT):
                nc.sync.dma_start(out=x_sb[:, j, :], in_=x_d[j * P:(j + 1) * P, :])

            # ones column of s_aug
            nc.vector.memset(s_sb[:, :, K:K + 1], 1.0)

            # ---------------- Phase S: s = softmax(x @ W^T + b) * m ----------
            with tc.tile_pool(name="ps", bufs=3, space="PSUM") as ps_pool, \
                 tc.tile_pool(name="pxt", bufs=3, space="PSUM") as pxt_pool, \
                 tc.tile_pool(name="scr", bufs=3) as scr:
                for j in range(NT):
                    ps_t = ps_pool.tile([P, K], fp32, tag="ps_t")
                    for fc in range(FT):
                        xt_p = pxt_pool.tile([P, P], fp32, tag="xt_p")
                        nc.tensor.transpose(
                            xt_p[:, :], x_sb[:, j, fc * P:(fc + 1) * P], ident[:, :])
                        xt_s = scr.tile([P, P], fp32, tag="xt_s")
                        nc.vector.tensor_copy(xt_s[:, :], xt_p[:, :])
                        nc.tensor.matmul(
                            ps_t[:, :], xt_s[:, :], wt_sb[:, fc, :],
                            start=(fc == 0), stop=(fc == FT - 1))
                    spre = scr.tile([P, K], fp32, tag="spre")
                    nc.vector.tensor_add(spre[:, :], ps_t[:, :], bb_sb[:, :])
                    rmaxn = scr.tile([P, 1], fp32, tag="rmaxn")
                    nc.vector.reduce_max(out=rmaxn[:, :], in_=spre[:, :], axis=AX.X,
                                         negate=True)
                    rsum = scr.tile([P, 1], fp32, tag="rsum")
                    nc.scalar.activation(
                        out=s_sb[:, j, 0:K], in_=spre[:, :], func=AF.Exp,
                        bias=rmaxn[:, 0:1], scale=1.0, accum_out=rsum[:, 0:1])
                    rinv = scr.tile([P, 1], fp32, tag="rinv")
                    nc.vector.reciprocal(rinv[:, :], rsum[:, :])
                    scl = scr.tile([P, 1], fp32, tag="scl")
                    nc.vector.tensor_mul(scl[:, :], rinv[:, :], m_sb[:, j:j + 1])
                    nc.vector.tensor_scalar_mul(
                        s_sb[:, j, 0:K], s_sb[:, j, 0:K], scl[:, 0:1])
                    sq = scr.tile([P, K], fp32, tag="sq")
                    nc.scalar.activation(
                        out=sq[:, :], in_=s_sb[:, j, 0:K], func=AF.Square,
                        accum_out=s_sb[:, j, K + 1:K + 2])
                    nc.vector.tensor_copy(s_bf[:, j, :], s_sb[:, j, :])

            # ---------------- Phase T + epilogue matmuls ---------------------
            with tc.tile_pool(name="po", bufs=1, space="PSUM") as po_pool, \
                 tc.tile_pool(name="pss", bufs=1, space="PSUM") as pss_pool, \
                 tc.tile_pool(name="pt", bufs=2, space="PSUM") as pt_pool, \
                 tc.tile_pool(name="ptr", bufs=2, space="PSUM") as ptr_pool, \
                 tc.tile_pool(name="poa", bufs=1, space="PSUM") as poa_pool:
                # out = s^T x and ss = s_aug^T s_aug — independent of A
                po_t = po_pool.tile([P, F], fp32, tag="po_t")
                for c in range(NT):
                    nc.tensor.matmul(po_t[:K2, :], s_sb[:, c, :], x_sb[:, c, :],
                                     start=(c == 0), stop=(c == NT - 1))
                pss_t = pss_pool.tile([P, K2], fp32, tag="pss_t")
                for c in range(NT):
                    nc.tensor.matmul(pss_t[:K2, :], s_sb[:, c, :], s_sb[:, c, :],
                                     start=(c == 0), stop=(c == NT - 1))
                nc.vector.tensor_copy(out_st[:K, :], po_t[0:K, :])
                nc.sync.dma_start(out=out_pre_d[:, :], in_=out_st[:K, :])
                nc.vector.tensor_copy(ss_st[:K2, :], pss_t[0:K2, :])
                nc.sync.dma_start(out=ss_d[:, :], in_=ss_st[:K2, :K2])

                # t = s_aug^T A, streaming A in column blocks of LBLK
                for lt in range(LTN):
                    ablk = apool.tile([P, NT, LBLK], adt, tag="ablk")
                    for c in range(NT):
                        nc.sync.dma_start(
                            out=ablk[:, c, :],
                            in_=a_d[c * P:(c + 1) * P, lt * LBLK:(lt + 1) * LBLK])
                    pt_t = pt_pool.tile([P, LBLK], fp32, tag="pt_t")
                    for c in range(NT):
                        nc.tensor.matmul(pt_t[:K2, :], s_bf[:, c, :], ablk[:, c, :],
                                         start=(c == 0), stop=(c == NT - 1))
                    nc.vector.tensor_copy(
                        t_sb[:K2, lt * LBLK:(lt + 1) * LBLK], pt_t[:K2, :])

                # tt = t^T  (N x K2), then oa_aug = tt^T' ... = s_aug^T A s_aug
                for j in range(NT):
                    tr_p = ptr_pool.tile([P, K2], fp32, tag="tr_p")
                    nc.tensor.transpose(
                        tr_p[:, :], t_sb[:K2, j * P:(j + 1) * P], ident[:K2, :K2])
                    nc.vector.tensor_copy(tt_sb[:, j, :], tr_p[:, :])
                poa_t = poa_pool.tile([P, K2], fp32, tag="poa_t")
                for c in range(NT):
                    nc.tensor.matmul(poa_t[:K2, :], tt_sb[:, c, :], s_sb[:, c, :],
                                     start=(c == 0), stop=(c == NT - 1))
                nc.vector.tensor_copy(oa_st[:K2, :], poa_t[0:K2, :])
                nc.sync.dma_start(out=oa_d[:, :], in_=oa_st[:K2, :K2])

    nc.compile()
    return nc


def _get_program():
    if "nc" not in _CACHE:
        _CACHE["nc"] = _build_program()
    return _CACHE["nc"]


def kernel(x_nodes, adj_matrix, x_masks, W, b, gamma, beta):
    from concourse.bass_utils import run_bass_kernel_spmd

    nc = _get_program()

    adt = ml_dtypes.bfloat16 if A_DTYPE == "bfloat16" else np.float32
    wt = np.ascontiguousarray(np.asarray(W, np.float32).T)            # [F, K]
    b_np = np.ascontiguousarray(np.asarray(b, np.float32))
    in_maps = []
    for i in range(B):
        in_maps.append({
            "x": np.ascontiguousarray(np.asarray(x_nodes[i], np.float32)),
            "a": np.ascontiguousarray(np.asarray(adj_matrix[i]).astype(adt)),
            "wt": wt,
            "b": b_np,
            "m": np.ascontiguousarray(np.asarray(x_masks[i], np.float32)),
        })
    res = run_bass_kernel_spmd(nc, in_maps, core_ids=list(range(B))).results

    out_pre = np.stack([res[i]["out_pre"] for i in range(B)])          # [B,K,F]
    oa = np.stack([res[i]["oa"] for i in range(B)])                    # [B,K2,K2]
    ss_aug = np.stack([res[i]["ss"] for i in range(B)])                # [B,K2,K2]

    # ---- host epilogue (tiny) ----
    out_adj = oa[:, :K, :K].astype(np.float64)
    mincut_num = np.trace(out_adj, axis1=1, axis2=2)
    mincut_den = oa[:, K + 1, K].astype(np.float64)
    mc = np.mean(-(mincut_num / mincut_den))

    ss = ss_aug[:, :K, :K].astype(np.float64)
    ss_fro = np.sqrt(np.sum(ss * ss, axis=(1, 2), keepdims=True))
    i_s = np.eye(K) / np.sqrt(K)
    o = np.mean(np.sqrt(np.sum((ss / ss_fro - i_s[None]) ** 2, axis=(1, 2))))

    eye = np.eye(K, dtype=np.float64)
    out_adj = out_adj * (1.0 - eye)
    d = np.sqrt(out_adj.sum(-1))[:, None, :] + EPS
    out_adj = out_adj / d / np.swapaxes(d, 1, 2)

    outf = out_pre.astype(np.float64)
    mean = outf.mean(axis=(0, 2), keepdims=True)
    var = outf.var(axis=(0, 2), keepdims=True)
    g = np.asarray(gamma, np.float64)[None, :, None]
    be = np.asarray(beta, np.float64)[None, :, None]
    x_af_pool = g * (outf - mean) / np.sqrt(var + BN_EPS) + be

    return (x_af_pool.astype(np.float32),
            out_adj.astype(np.float32),
            np.float32(mc + o))
